# revision 60
# baseline (speedup 1.0000x reference)
"""GQA kernel for Trainium2, 8 NeuronCores.

Problem: x[2,2048,2048] -> GQA(16 heads, 4 kv groups, dk=128) -> out[2,2048,2048]

Sharding: core c handles (batch b = c//4, kv-group g = c%4), i.e. the 4 query
heads of one group on one batch. Zero replication of FLOPs across cores:
per-core work = Qproj(4 heads) + K/Vproj(1 group) + attention(4 heads) +
row-slice of the O projection. Host sums the 4 per-group partial outputs
per batch (the row-parallel O-proj reduction) and adds bo.

All matmuls run in bf16 (full PE rate, half the DMA/SBUF of f32), psum f32.
Inputs are converted to bf16 on the host; the partial output returns as bf16
and is reduced in f32 on the host.

On-core dataflow (all matmuls contract over the partition dim):
  stage 1 (per 512-wide seq chunk j): xT chunk [d,512] streams in;
    K/V projections -> ktT/vtT bf16; V transposed via PE into [s,dk|1]
    chunks (ones column gives the softmax denominator for free);
    Q projections (chunks 0..2) -> qtT per head.
  stage 2 (per chunk j, per head h): 8x { 2 score matmuls [sk,512] into a
    2-bank psum pair; one exp over [128,2x512] (ACT) -> bf16 attn pair-tile }
    interleaved, via a filler deque, with: AV matmuls of the previous head
    (attn.T @ [V|1], denominator normalize on DVE, PE transpose back to
    [dk,q]), O-projection groups of the previous chunk, and the deferred
    Q-projection of chunk 3. The deque keeps PE busy while ACT exps run.
"""

import math
from collections import deque

import numpy as np
import ml_dtypes

import concourse.bass as bass
import concourse.mybir as mybir
import concourse.tile as tile
from concourse import bacc
from concourse.bass_utils import run_bass_kernel_spmd

F32 = mybir.dt.float32
BF16 = mybir.dt.bfloat16

D = 2048          # d_model
S = 2048          # seq len
DK = 128          # head dim
HPG = 4           # heads per kv group
QCOLS = HPG * DK  # 512 q columns per core
N_CORES = 8
SCALE = 1.0 / math.sqrt(DK)

SJ = 512                    # seq chunk (free dim of proj/scores matmuls)
NJ = S // SJ                # 4 chunks
NSK = S // 128              # 16 key chunks
ND = D // 128               # 16 d_model chunks
NSUB = SJ // 128            # 4 q sub-tiles per chunk
NDC = D // 512              # 4 output column chunks


def build_program(n_reps=1):
    nc = bacc.Bacc("TRN2", target_bir_lowering=False, debug=False,
                   num_devices=N_CORES)

    xt = nc.dram_tensor("xt", [D, S], BF16, kind="ExternalInput").ap()
    wq = nc.dram_tensor("wq", [D, QCOLS], BF16, kind="ExternalInput").ap()
    wk = nc.dram_tensor("wk", [D, DK], BF16, kind="ExternalInput").ap()
    wv = nc.dram_tensor("wv", [D, DK], BF16, kind="ExternalInput").ap()
    wo = nc.dram_tensor("wo", [QCOLS, D], BF16, kind="ExternalInput").ap()
    bq = nc.dram_tensor("bq", [QCOLS], F32, kind="ExternalInput").ap()
    bk = nc.dram_tensor("bk", [DK], F32, kind="ExternalInput").ap()
    bv = nc.dram_tensor("bv", [DK], F32, kind="ExternalInput").ap()
    out = nc.dram_tensor("out", [S, D], BF16, kind="ExternalOutput").ap()

    with tile.TileContext(nc) as tc:
      for _rep in range(n_reps):
        with (
            tc.tile_pool(name="singles", bufs=1) as singles,
            tc.tile_pool(name="xp", bufs=3) as xpool,
            tc.tile_pool(name="vtp", bufs=2) as vtpool,
            tc.tile_pool(name="attn", bufs=36) as attnpool,
            tc.tile_pool(name="aot", bufs=36) as aotpool,
            tc.tile_pool(name="osb", bufs=12) as outpool,
            tc.tile_pool(name="small", bufs=6) as smallpool,
            tc.tile_pool(name="psSC", bufs=4, space="PSUM") as psSC,
            tc.tile_pool(name="psAV", bufs=2, space="PSUM") as psAV,
            tc.tile_pool(name="psO", bufs=2, space="PSUM") as psO,
        ):
            # ---- activation table + PE p-state warmup (runs during the
            # initial DMAs; the PE clock ramps with sustained use) ----
            warm = singles.tile([128, 1], F32)
            nc.vector.memset(warm, 0.0)
            warm_o = singles.tile([128, 1], BF16)
            nc.scalar.activation(out=warm_o, in_=warm,
                                 func=mybir.ActivationFunctionType.Exp)
            warm_mm = singles.tile([128, 128], BF16)
            nc.vector.memset(warm_mm, 0.0)
            for w in range(30):
                pw = psAV.tile([128, 128], F32, tag="av", name=f"pw{w}")
                nc.tensor.matmul(pw, lhsT=warm_mm, rhs=warm_mm,
                                 start=True, stop=True)

            # ---- resident weights / biases (chunked DMAs, K/V first;
            # wq/wo/biases interleaved behind the xt chunk DMAs) ----
            wk_sb = singles.tile([128, ND, DK], BF16)
            wk_r = wk.rearrange("(c p) n -> p c n", p=128)
            nc.sync.dma_start(out=wk_sb[:, 0:ND // 2, :],
                              in_=wk_r[:, 0:ND // 2, :])
            wv_sb = singles.tile([128, ND, DK], BF16)
            wv_r = wv.rearrange("(c p) n -> p c n", p=128)
            bias_sb = singles.tile([128, HPG + 2], F32)
            bq_sb = bias_sb[:, 0:HPG]
            bk_sb = bias_sb[:, HPG:HPG + 1]
            bv_sb = bias_sb[:, HPG + 1:HPG + 2]
            wq_sb = singles.tile([128, ND, QCOLS], BF16)
            wq_r = wq.rearrange("(c p) n -> p c n", p=128)
            wo_sb = singles.tile([128, HPG, D], BF16)
            wo_r = wo.rearrange("(h p) n -> p h n", p=128)

            def late_dmas(j):
                # biases + weight chunks interleaved behind each xt chunk,
                # in the order stage 1/2 consume them
                if j == 0:
                    nc.sync.dma_start(
                        out=bias_sb[:, 0:HPG],
                        in_=bq.rearrange("(h p) -> p h", p=128))
                    nc.sync.dma_start(out=bias_sb[:, HPG:HPG + 1],
                                      in_=bk.unsqueeze(1))
                    nc.sync.dma_start(out=bias_sb[:, HPG + 1:HPG + 2],
                                      in_=bv.unsqueeze(1))
                elif j < 3:
                    h2 = j - 1
                    nc.sync.dma_start(out=wo_sb[:, 2 * h2:2 * h2 + 2, :],
                                      in_=wo_r[:, 2 * h2:2 * h2 + 2, :])

            qt_sb = singles.tile([128, HPG, S], BF16)    # QT per head [dk, S]
            kt_sb = singles.tile([128, S], BF16)         # KT [dk, S]
            # [V | 1] per key chunk; separate tiles because the XBAR DMA
            # transpose needs an offset-0 destination
            vones = []
            for k in range(NSK):
                vk = singles.tile([128, 132], BF16, tag=f"von{k}",
                                  name=f"vones{k}")
                nc.vector.memset(vk[:, 128:129], 1.0)
                vones.append(vk)

            xts = [None] * NJ

            def xread(j, d):
                if j == 0:
                    return xts[0][d // 2][:, d % 2, :]
                return xts[j][:, d, :]

            def qproj(j, h, pq=None):
                """Q projection for (chunk j, head h) into a psum slot."""
                if pq is None:
                    pq = psSC.tile([128, SJ], F32, tag="sc", name=f"pq{j}_{h}")
                for d in range(ND):
                    nc.tensor.matmul(
                        pq, lhsT=wq_sb[:, d, bass.ts(h, 128)],
                        rhs=xread(j, d),
                        start=(d == 0), stop=(d == ND - 1))
                nc.vector.tensor_scalar_add(
                    qt_sb[:, h, bass.ts(j, SJ)], pq, bq_sb[:, h:h + 1])

            # ---- stage 1: K/V (+V transpose) all chunks; Q for chunks 0..2
            for j in range(NJ):
                xt_r = xt[:, bass.ts(j, SJ)].rearrange("(c p) s -> p c s", p=128)
                if j == 0:
                    # chunk 0 is DMA-bound: stream x in quarters with the
                    # weight chunks paced between them, and keep all six
                    # projection groups (K, V, Q0-Q3) open at once so every
                    # arriving quarter immediately feeds 24 matmuls
                    xqs = []
                    pk = psSC.tile([128, SJ], F32, tag="sc")
                    pv = psSC.tile([128, SJ], F32, tag="sc")
                    pqs = [psSC.tile([128, SJ], F32, tag="sc", name="pq0"),
                           psSC.tile([128, SJ], F32, tag="sc", name="pq1"),
                           psO.tile([128, SJ], F32, tag="o", name="pq2"),
                           psO.tile([128, SJ], F32, tag="o", name="pq3")]
                    for e in range(8):
                        xq = xpool.tile([128, 2, SJ], BF16, tag="xq",
                                        bufs=8, name=f"xq{e}")
                        nc.sync.dma_start(
                            out=xq, in_=xt_r[:, 2 * e:2 * e + 2, :])
                        xqs.append(xq)
                        if e == 0:
                            nc.sync.dma_start(out=wv_sb[:, 0:ND // 2, :],
                                              in_=wv_r[:, 0:ND // 2, :])
                        elif e == 2:
                            nc.sync.dma_start(out=wk_sb[:, ND // 2:, :],
                                              in_=wk_r[:, ND // 2:, :])
                            nc.sync.dma_start(out=wv_sb[:, ND // 2:, :],
                                              in_=wv_r[:, ND // 2:, :])
                        if e % 2 == 1:
                            c4 = e // 2
                            nc.sync.dma_start(
                                out=wq_sb[:, 4 * c4:4 * c4 + 4, :],
                                in_=wq_r[:, 4 * c4:4 * c4 + 4, :])
                    xts[0] = xqs
                    late_dmas(0)
                    for q in range(4):
                        qd = range(4 * q, 4 * q + 4)
                        for d in qd:
                            nc.tensor.matmul(pk, lhsT=wk_sb[:, d, :],
                                             rhs=xread(0, d),
                                             start=(d == 0), stop=(d == ND - 1),
                                             skip_group_check=True)
                        for d in qd:
                            nc.tensor.matmul(pv, lhsT=wv_sb[:, d, :],
                                             rhs=xread(0, d),
                                             start=(d == 0), stop=(d == ND - 1),
                                             skip_group_check=True)
                        for h in range(HPG):
                            for d in qd:
                                nc.tensor.matmul(
                                    pqs[h], lhsT=wq_sb[:, d, bass.ts(h, 128)],
                                    rhs=xread(0, d),
                                    start=(d == 0), stop=(d == ND - 1),
                                    skip_group_check=True)
                    nc.vector.tensor_scalar_add(
                        kt_sb[:, bass.ts(0, SJ)], pk, bk_sb)
                    vt_sb = vtpool.tile([128, SJ], BF16)
                    nc.vector.tensor_scalar_add(vt_sb, pv, bv_sb)
                    for sub in range(NSUB):
                        nc.sync.dma_start_transpose(
                            out=vones[sub][:, 0:128],
                            in_=vt_sb[:, bass.ts(sub, 128)])
                    for h in range(HPG):
                        nc.vector.tensor_scalar_add(
                            qt_sb[:, h, bass.ts(0, SJ)], pqs[h],
                            bq_sb[:, h:h + 1])
                    continue

                xt_sb = xpool.tile([128, ND, SJ], BF16)
                xts[j] = xt_sb
                nc.sync.dma_start(out=xt_sb, in_=xt_r)
                late_dmas(j)

                pk = psSC.tile([128, SJ], F32, tag="sc")
                for d in range(ND):
                    nc.tensor.matmul(pk, lhsT=wk_sb[:, d, :],
                                     rhs=xread(j, d),
                                     start=(d == 0), stop=(d == ND - 1))
                nc.vector.tensor_scalar_add(
                    kt_sb[:, bass.ts(j, SJ)], pk, bk_sb)
                pv = psSC.tile([128, SJ], F32, tag="sc")
                for d in range(ND):
                    nc.tensor.matmul(pv, lhsT=wv_sb[:, d, :],
                                     rhs=xread(j, d),
                                     start=(d == 0), stop=(d == ND - 1))
                vt_sb = vtpool.tile([128, SJ], BF16)
                nc.vector.tensor_scalar_add(vt_sb, pv, bv_sb)
                # VT [dk,512] -> V [s,dk] chunks via XBAR DMA transpose
                for sub in range(NSUB):
                    nc.sync.dma_start_transpose(
                        out=vones[j * NSUB + sub][:, 0:128],
                        in_=vt_sb[:, bass.ts(sub, 128)])
                if j < NJ - 1:
                    for h in range(HPG):
                        qproj(j, h)

            # ---- stage 2: attention + O-projection, filler-deque interleave
            fillers = deque()
            aots = {}

            def make_av(j, h, sub, attns):
                def emit():
                    pav = psAV.tile([128, 132], F32, tag="av")
                    for sk in range(NSK):
                        nc.tensor.matmul(
                            pav[:, 0:129],
                            lhsT=attns[sk][:, bass.ts(sub, 128)],
                            rhs=vones[sk][:, 0:129],
                            start=(sk == 0), stop=(sk == NSK - 1))
                    recip = smallpool.tile([128, 1], F32, tag="recip")
                    nc.vector.reciprocal(recip, pav[:, 128:129])
                    ao = smallpool.tile([128, 128], BF16, tag="ao")
                    nc.vector.tensor_scalar_mul(ao, pav[:, 0:128], recip)
                    at = aotpool.tile([128, 128], BF16, tag="aot",
                                      name=f"aot{j}_{h}_{sub}")
                    aots[(j, h, sub)] = at
                    nc.sync.dma_start_transpose(out=at, in_=ao)
                return emit

            osbs = {}

            def make_oproj(j, sub, dc):
                def emit():
                    po = psO.tile([128, 512], F32, tag="o")
                    for h in range(HPG):
                        nc.tensor.matmul(
                            po, lhsT=aots[(j, h, sub)],
                            rhs=wo_sb[:, h, bass.ts(dc, 512)],
                            start=(h == 0), stop=(h == HPG - 1))
                    if j == NJ - 1:
                        # tail: all copies on ACT (exps done, DVE must stay
                        # free for the AV normalize chain), and batch the 4
                        # column chunks into one DMA to cut queue overhead
                        key = (j, sub)
                        if key not in osbs:
                            osbs[key] = [outpool.tile(
                                [128, D], BF16, tag="osbw", bufs=4,
                                name=f"osbw{sub}"), 0]
                        osb, _ = osbs[key]
                        nc.scalar.activation(
                            out=osb[:, bass.ts(dc, 512)], in_=po,
                            func=mybir.ActivationFunctionType.Identity)
                        osbs[key][1] += 1
                        if osbs[key][1] == NDC:
                            nc.sync.dma_start(
                                out=out[j * SJ + sub * 128:
                                        j * SJ + (sub + 1) * 128, :],
                                in_=osb)
                    else:
                        osb = outpool.tile([128, 512], BF16, tag="osb",
                                           name=f"osb{j}_{sub}_{dc}")
                        nc.vector.tensor_copy(osb, po)
                        nc.sync.dma_start(
                            out=out[j * SJ + sub * 128:
                                    j * SJ + (sub + 1) * 128,
                                    bass.ts(dc, 512)],
                            in_=osb)
                emit.rotatable = True
                return emit

            # deferred Q projection of the last chunk, split into 4-deep
            # accumulation pieces so it spreads across chunk 0's slots
            qp3_psum = {}

            def make_qproj3(h, piece):
                def emit():
                    if piece == 0:
                        qp3_psum[h] = psO.tile([128, 512], F32, tag="o",
                                               name=f"qp3_{h}")
                    pq = qp3_psum[h]
                    for d in range(piece * 4, piece * 4 + 4):
                        nc.tensor.matmul(
                            pq, lhsT=wq_sb[:, d, bass.ts(h, 128)],
                            rhs=xts[NJ - 1][:, d, :],
                            start=(d == 0), stop=(d == ND - 1),
                            skip_group_check=True)
                    if piece == 3:
                        nc.vector.tensor_scalar_add(
                            qt_sb[:, h, bass.ts(NJ - 1, SJ)], pq,
                            bq_sb[:, h:h + 1])
                return emit

            for h in range(HPG):
                for piece in range(4):
                    fillers.append(make_qproj3(h, piece))

            for j in range(NJ):
                for h in range(HPG):
                    attns = []
                    for sk in range(NSK):
                        ps = psSC.tile([128, SJ], F32, tag="sc")
                        nc.tensor.matmul(
                            ps, lhsT=kt_sb[:, bass.ts(sk, 128)],
                            rhs=qt_sb[:, h, bass.ts(j, SJ)],
                            start=True, stop=True)
                        a = attnpool.tile([128, SJ], BF16)
                        nc.scalar.activation(
                            out=a, in_=ps,
                            func=mybir.ActivationFunctionType.Exp, scale=SCALE)
                        attns.append(a)
                        if sk % 2 == 1 and fillers:
                            if j == NJ - 1 and h == HPG - 1 and sk >= 12:
                                # reserve two independent O groups to cover
                                # the tail's wait for this head's last exps
                                continue
                            fillers.popleft()()
                    # AV of this head: drained during the next head's scores
                    leftovers = []
                    if j == NJ - 1 and h == HPG - 1:
                        # the reserved O groups go ahead of the final AVs
                        leftovers = list(fillers)
                        fillers.clear()
                    for sub in reversed(range(NSUB)):
                        fillers.appendleft(make_av(j, h, sub, attns))
                    for it in reversed(leftovers):
                        fillers.appendleft(it)
                    if h < HPG - 1:
                        # rotate one independent back item ahead of the AVs so
                        # the first filler slot of the next head does not wait
                        # on this head's last exps (invalid at h==3: O-proj of
                        # this chunk must stay behind AV of head 3; qproj3
                        # pieces are order-dependent and must not rotate)
                        if len(fillers) > NSUB and getattr(
                                fillers[-1], "rotatable", False):
                            fillers.appendleft(fillers.pop())
                    else:
                        for sub in range(NSUB):
                            for dc in range(NDC):
                                fillers.append(make_oproj(j, sub, dc))
            # tail drain: all AVs first (their transposes stream out on the
            # ACT queue while the O matmuls start), then the O groups
            while fillers:
                fillers.popleft()()

    nc.compile()
    return nc


_NC_CACHE = None


def _get_program():
    global _NC_CACHE
    if _NC_CACHE is None:
        _NC_CACHE = build_program()
    return _NC_CACHE


def kernel(x, Wq, bq, Wk, bk, Wv, bv, Wo, bo):
    bf16 = ml_dtypes.bfloat16
    x = np.asarray(x, np.float32)
    nc = _get_program()

    in_maps = []
    xts = [np.ascontiguousarray(x[b].T).astype(bf16) for b in range(x.shape[0])]
    Wq = np.asarray(Wq, np.float32)
    Wk = np.asarray(Wk, np.float32)
    Wv = np.asarray(Wv, np.float32)
    Wo = np.asarray(Wo, np.float32)
    for c in range(N_CORES):
        b, g = divmod(c, HPG)
        in_maps.append({
            "xt": xts[b],
            "wq": np.ascontiguousarray(Wq[:, g * QCOLS:(g + 1) * QCOLS]).astype(bf16),
            "wk": np.ascontiguousarray(Wk[:, g * DK:(g + 1) * DK]).astype(bf16),
            "wv": np.ascontiguousarray(Wv[:, g * DK:(g + 1) * DK]).astype(bf16),
            "wo": np.ascontiguousarray(Wo[g * QCOLS:(g + 1) * QCOLS, :]).astype(bf16),
            "bq": np.ascontiguousarray(np.asarray(bq, np.float32)[g * QCOLS:(g + 1) * QCOLS]),
            "bk": np.ascontiguousarray(np.asarray(bk, np.float32)[g * DK:(g + 1) * DK]),
            "bv": np.ascontiguousarray(np.asarray(bv, np.float32)[g * DK:(g + 1) * DK]),
        })

    res = run_bass_kernel_spmd(nc, in_maps, core_ids=list(range(N_CORES))).results

    outv = np.zeros((x.shape[0], S, D), np.float32)
    for c in range(N_CORES):
        b = c // HPG
        outv[b] += np.asarray(res[c]["out"], dtype=np.float32)
    outv += np.asarray(bo, np.float32)
    return outv
